# revision 1
# baseline (speedup 1.0000x reference)
"""ChebNet (K=5, 5 conv layers, H=48) forward on 8 TRN2 NeuronCores.

Host prep: relabel nodes (in-degree sorted, round-robin over cores; the output
is graph-pooled so relabeling is free), build per-node in-edge slot tables
padded to per-group quantized degrees, precompute wedge/diag.

Device per prop (20 total): indirect-DMA gather of src rows (192B each) from a
replicated HBM table -> DVE wedge-multiply + slot tensor_reduce -> Chebyshev
combine -> AllGather local slices into the next table.  PE transposes each
T_k to feature-major and accumulates sum_k T_k @ W_k; ACT does PSUM->SBUF
copies and ReLU.  Pooling = one-hot matmul; FCs on device; out = [G,1].
"""

import contextlib
import numpy as np
from dataclasses import dataclass, field

P = 128
QLEV = [4, 6, 8, 12, 16, 24, 32, 48, 64, 96, 128, 192, 256, 384, 512, 768, 1024]


def _quant(d):
    for q in QLEV:
        if q >= d:
            return q
    raise ValueError(d)


@dataclass
class Meta:
    N: int; E: int; G: int; F: int; H: int; K: int; L: int
    ncores: int; NLOC: int; NGRP: int; NLOCP: int; NT: int
    Dg: list; colst: list; TD: int; CMAX: int
    tiles: list = field(default_factory=list)
    chunks: list = field(default_factory=list)
    FC1: int = 32
    CH: int = 32768
    NCHK: int = 4
    gsegs: list = field(default_factory=list)
    toks_per_tile: list = field(default_factory=list)
    TOTTOK: int = 0


def make_tiles(meta: Meta):
    tiles = []
    g = 0
    while g < meta.NGRP:
        c0 = meta.colst[g]
        g1 = g
        while g1 < meta.NGRP and meta.colst[g1 + 1] - c0 <= meta.CMAX:
            g1 += 1
        if g1 == g:       # single oversized group gets its own tile
            g1 = g + 1
        c1 = meta.colst[g1]
        segs = []
        gg = g
        while gg < g1:
            ge = gg
            while ge < g1 and meta.Dg[ge] == meta.Dg[gg]:
                ge += 1
            segs.append((meta.colst[gg] - c0, gg, ge - gg, meta.Dg[gg]))
            gg = ge
        tiles.append((c0, c1, segs))
        g = g1
    meta.tiles = tiles
    chunks = []
    g = 0
    while g < meta.NGRP:
        n = min(4, meta.NGRP - g)
        chunks.append((g, n))
        g += n
    meta.chunks = chunks


def preprocess(x, edge_index, batch, lmax, ncores=8, cmax=128):
    x = np.asarray(x, np.float32)
    src = np.asarray(edge_index[0], np.int64)
    dst = np.asarray(edge_index[1], np.int64)
    batch = np.asarray(batch, np.int64)
    lmax = np.asarray(lmax, np.float32)
    N, F = x.shape
    E = src.shape[0]
    G = lmax.shape[0]

    deg = np.bincount(src, minlength=N).astype(np.float32)
    dis = np.where(deg > 0,
                   1.0 / np.sqrt(np.where(deg > 0, deg, 1.0).astype(np.float32)),
                   0.0).astype(np.float32)
    wedge = (-2.0 * dis[src] * dis[dst] / lmax[batch[src]]).astype(np.float32)
    diag = (2.0 / lmax[batch] - 1.0).astype(np.float32)

    indeg = np.bincount(dst, minlength=N)
    order = np.argsort(-indeg, kind="stable")
    NLOC = (N + ncores - 1) // ncores
    NGRP = (NLOC + P - 1) // P
    NLOCP = NGRP * P
    NT = ncores * NLOCP
    j = np.arange(N)
    new_id = np.empty(N, np.int64)
    new_id[order] = (j % ncores) * NLOCP + j // ncores
    indeg_sorted = indeg[order]

    src_n = new_id[src]
    dst_n = new_id[dst]

    Dg = []
    for g in range(NGRP):
        kq = g * P * ncores
        dmax = int(indeg_sorted[min(kq, N - 1)]) if kq < N else 0
        Dg.append(max(2, ((dmax + 1) // 2) * 2))
    colst = np.concatenate([[0], np.cumsum(Dg)]).astype(np.int64)
    TD = int(colst[-1])

    c_e = dst_n // NLOCP
    ldst = dst_n % NLOCP
    key = c_e * NLOCP + ldst
    ord2 = np.argsort(key, kind="stable")
    sk = key[ord2]
    starts = np.flatnonzero(np.concatenate([[True], sk[1:] != sk[:-1]]))
    counts = np.diff(np.concatenate([starts, [E]]))
    slot = np.arange(E) - np.repeat(starts, counts)
    g_of = (sk % NLOCP) // P
    p_of = sk % P
    c_of = sk // NLOCP
    assert (slot < np.array(Dg)[g_of]).all()
    col = colst[g_of] + slot

    idxa_arr = np.zeros((ncores, P, TD), np.int32)
    wedge_arr = np.zeros((ncores, P, TD), np.float32)
    idxa_arr[c_of, p_of, col] = src_n[ord2].astype(np.int32)
    wedge_arr[c_of, p_of, col] = wedge[ord2]

    diag_arr = np.zeros((ncores, P, NGRP), np.float32)
    cj, rj = j % ncores, j // ncores
    diag_arr[cj, rj % P, rj // P] = diag[order]

    x_table = np.zeros((NT, F), np.float32)
    x_table[new_id] = x
    # x local, node-major blocks: [P, NGRP*F], block g col f = x of node g*128+p
    xnm = np.zeros((ncores, P, NGRP * F), np.float32)
    xl = x_table.reshape(ncores, NGRP, P, F)
    xnm[:] = xl.transpose(0, 2, 1, 3).reshape(ncores, P, NGRP * F)

    bmat = np.zeros((ncores, NLOCP, G), np.float32)
    bnew = np.full(NT, -1, np.int64)
    bnew[new_id] = batch
    for c in range(ncores):
        bl = bnew[c * NLOCP:(c + 1) * NLOCP]
        msk = bl >= 0
        bmat[c][np.flatnonzero(msk), bl[msk]] = 1.0

    meta = Meta(N=N, E=E, G=G, F=F, H=48, K=5, L=5, ncores=ncores, NLOC=NLOC,
                NGRP=NGRP, NLOCP=NLOCP, NT=NT, Dg=Dg, colst=list(map(int, colst)),
                TD=TD, CMAX=cmax)
    make_tiles(meta)
    arrs = dict(idx=idxa_arr, wedge=wedge_arr, diag=diag_arr,
                x_table=x_table, xnm=xnm, bmat=bmat)
    return meta, arrs


def pack_weights(meta, W1, b1, W2, b2, W3, b3, W4, b4, W5, b5,
                 fc1_w, fc1_b, fc2_w, fc2_b):
    K, F, H = np.asarray(W1).shape
    w1p = np.ascontiguousarray(np.asarray(W1, np.float32).transpose(1, 0, 2).reshape(F, K * H))
    w2p = np.concatenate(
        [np.asarray(W, np.float32).transpose(1, 0, 2).reshape(H, K * H)
         for W in (W2, W3, W4, W5)], axis=1)
    bp = np.stack([np.asarray(b, np.float32) for b in (b1, b2, b3, b4, b5)], axis=1)
    return dict(w1=w1p, w2=w2p, bias=bp,
                fc1w=np.asarray(fc1_w, np.float32),
                fc1b=np.asarray(fc1_b, np.float32).reshape(-1, 1),
                fc2w=np.asarray(fc2_w, np.float32).reshape(-1, 1),
                fc2b=np.asarray(fc2_b, np.float32).reshape(1, 1))


# ------------------------------------------------------------------ numpy model
def emulate(meta, arrs, wts):
    m = meta
    idx, wedge, diag = arrs["idx"], arrs["wedge"], arrs["diag"]
    table = arrs["x_table"]
    h_nm = None   # [nc, NLOCP, Fin] local node-major (natural order)
    x_loc = table.reshape(m.ncores, m.NLOCP, m.F)
    h_loc = x_loc
    h = None
    for l in range(m.L):
        Fin = m.F if l == 0 else m.H
        wl = wts["w1"] if l == 0 else wts["w2"][:, (l - 1) * m.K * m.H:l * m.K * m.H]
        out = np.einsum("cnf,fh->cnh", h_loc, wl[:, 0:m.H])
        t_loc = h_loc
        t_prev = None
        cur_table = table
        for k in range(1, m.K):
            gath = cur_table[idx]                              # [nc, P, TD, Fin]
            Y = gath * wedge[..., None]
            red = np.zeros((m.ncores, P, m.NGRP, Fin), np.float32)
            for g in range(m.NGRP):
                red[:, :, g] = Y[:, :, m.colst[g]:m.colst[g + 1]].sum(2)
            tl4 = t_loc.reshape(m.ncores, m.NGRP, P, Fin).transpose(0, 2, 1, 3)
            prop = red + diag[..., None] * tl4
            if k == 1:
                t_new4 = prop
            else:
                tp4 = t_prev.reshape(m.ncores, m.NGRP, P, Fin).transpose(0, 2, 1, 3)
                t_new4 = 2.0 * prop - tp4
            t_new = np.ascontiguousarray(t_new4.transpose(0, 2, 1, 3)).reshape(
                m.ncores, m.NLOCP, Fin)
            out += np.einsum("cnf,fh->cnh", t_new, wl[:, k * m.H:(k + 1) * m.H])
            t_prev, t_loc = t_loc, t_new
            if k < m.K - 1:
                cur_table = t_new.reshape(m.NT, Fin)
        h = np.maximum(out + wts["bias"][:, l], 0.0)
        h_loc = h
        table = h.reshape(m.NT, m.H)
    pooled = np.einsum("cng,cnh->gh", arrs["bmat"], h)
    z = np.maximum(pooled @ wts["fc1w"] + wts["fc1b"].ravel(), 0.0)
    return (z @ wts["fc2w"] + wts["fc2b"].ravel()).astype(np.float32)


# ------------------------------------------------------------------ bass build
def build_nc(meta):
    import concourse.bass as bass
    import concourse.mybir as mybir
    from concourse.bass import IndirectOffsetOnAxis

    F32 = mybir.dt.float32
    BF16 = mybir.dt.bfloat16
    I32 = mybir.dt.int32
    RELU = mybir.ActivationFunctionType.Relu
    COPY = mybir.ActivationFunctionType.Copy
    ADD = mybir.AluOpType.add
    MULT = mybir.AluOpType.mult
    SUB = mybir.AluOpType.subtract
    X = mybir.AxisListType.X

    import concourse.bacc as bacc
    from concourse import library_config

    m = meta
    EW = 128
    nt = len(m.tiles)
    NCH = len(m.chunks)
    NPROP = m.L * (m.K - 1)
    core_ids = list(range(m.ncores))
    nc = bacc.Bacc("TRN2")

    x_table_p = nc.declare_dram_parameter("x_table", [m.NT, m.F], BF16, False)
    xnm_p = nc.declare_dram_parameter("xnm", [P, m.NGRP * m.F], F32, False)
    idx_p = nc.declare_dram_parameter("idx", [P, m.TD], I32, False)
    wedge_p = nc.declare_dram_parameter("wedge", [P, m.TD], BF16, False)
    diag_p = nc.declare_dram_parameter("diag", [P, m.NGRP], F32, False)
    bmat_p = nc.declare_dram_parameter("bmat", [m.NLOCP, m.G], F32, False)
    w1_p = nc.declare_dram_parameter("w1", [m.F, m.K * m.H], F32, False)
    w2_p = nc.declare_dram_parameter("w2", [m.H, (m.L - 1) * m.K * m.H], F32, False)
    bias_p = nc.declare_dram_parameter("bias", [m.H, m.L], F32, False)
    fc1w_p = nc.declare_dram_parameter("fc1w", [m.H, m.FC1], F32, False)
    fc1b_p = nc.declare_dram_parameter("fc1b", [m.FC1, 1], F32, False)
    fc2w_p = nc.declare_dram_parameter("fc2w", [m.FC1, 1], F32, False)
    fc2b_p = nc.declare_dram_parameter("fc2b", [1, 1], F32, False)
    ident_p = nc.declare_dram_parameter("ident", [P, P], F32, False)
    out_p = nc.declare_dram_parameter("out", [1, m.G], F32, True)
    DBG = bool(getattr(m, "debug", False))
    if DBG:
        dbg0_p = nc.declare_dram_parameter("dbg0", [P, m.NGRP * m.H], F32, True)
        dbg1_p = nc.declare_dram_parameter("dbg1", [P, m.NGRP * m.H], F32, True)
        dbg2_p = nc.declare_dram_parameter("dbg2", [m.H, m.G], F32, True)
    PRELOADS = 12

    tabs = [nc.dram_tensor(f"tab{i}", [m.NT, m.H], BF16, addr_space="Shared")
            for i in range(4)]
    slc = [nc.dram_tensor(f"slc{i}", [m.NLOCP, m.H], BF16) for i in range(2)]
    pool_in = nc.dram_tensor("pool_in", [m.H, m.G], F32)
    pool_red = nc.dram_tensor("pool_red", [m.H, m.G], F32, addr_space="Shared")

    ctx = contextlib.ExitStack()
    _cnt = [0]

    def sb(shape, dt=F32):
        _cnt[0] += 1
        return ctx.enter_context(nc.sbuf_tensor(f"sb{_cnt[0]}", shape, dt))

    def ps(shape):
        _cnt[0] += 1
        return ctx.enter_context(nc.psum_tensor(f"ps{_cnt[0]}", shape, F32))

    sb_idx = sb([P, m.TD], I32)
    sb_wedge = sb([P, m.TD], BF16)
    sb_diag = sb([P, m.NGRP])
    gmax = max(c1 - c0 for c0, c1, _ in m.tiles)
    gbuf = [sb([P, gmax, m.H], BF16) for _ in range(2)]
    nm = [sb([P, m.NGRP * m.H]) for _ in range(3)]
    nmh = sb([P, m.NGRP * m.H])
    xnm_sb = sb([P, m.NGRP * m.F])
    fm = sb([m.H, m.NLOCP])
    tkfm = [sb([m.H, 512]) for _ in range(2)]
    w1_sb = sb([m.F, m.K * m.H])
    w2_sb = sb([m.H, (m.L - 1) * m.K * m.H])
    bias_sb = sb([m.H, m.L])
    fc1w_sb = sb([m.H, m.FC1])
    fc1b_sb = sb([m.FC1, 1])
    fc2w_sb = sb([m.FC1, 1])
    fc2b_sb = sb([1, 1])
    ident_sb = sb([P, P])
    bt = [sb([P, m.G]) for _ in range(2)]
    pooled_sb = sb([m.H, m.G])
    fc1_sb = sb([m.FC1, m.G])
    out_sb = sb([1, m.G])

    psT = [ps([P, 512]) for _ in range(2)]
    psB = [ps([P, 512]) for _ in range(2)]
    psP = ps([P, 512])

    sem_gr = {f"g{p}_{r}": ctx.enter_context(nc.semaphore(f"sem_g{p}_{r}"))
              for p in (0, 1) for r in range(10)}
    sem_b0 = ctx.enter_context(nc.semaphore("sem_b0"))
    sem_b1 = ctx.enter_context(nc.semaphore("sem_b1"))
    sem_hw = ctx.enter_context(nc.semaphore("sem_hw"))
    sem_s = ctx.enter_context(nc.semaphore("sem_s"))
    sem_cc = ctx.enter_context(nc.semaphore("sem_cc"))
    sem_v = ctx.enter_context(nc.semaphore("sem_v"))
    sem_p = ctx.enter_context(nc.semaphore("sem_p"))
    sem_a = ctx.enter_context(nc.semaphore("sem_a"))
    SEMS = {"b0": sem_b0, "b1": sem_b1,
            "hw": sem_hw, "s": sem_s, "cc": sem_cc, "v": sem_v, "p": sem_p,
            "a": sem_a}
    SEMS.update(sem_gr)

    marks = {}
    VARIANT = getattr(m, "variant", "full")

    def emit(eng, which, record):
        n = dict(b0=0, b1=0, hw=0, s=0, cc=0, v=0, p=0, a=0)
        for _p in (0, 1):
            for _r in range(10):
                n[f"g{_p}_{_r}"] = 0

        def inc(kn, inst=None):
            amt = 16 if kn.startswith(("g", "b", "hw", "s")) else 1
            n[kn] += amt
            if inst is not None:
                inst.then_inc(SEMS[kn], amt)

        def wait(kn, val):
            if which and val > 0:
                eng.wait_ge(SEMS[kn], val)

        def mark(key):
            if record:
                marks[key] = dict(n)
            return marks[key]

        def mget(key, kn):
            if key not in marks:  # only during record pass for fwd refs
                return 0
            return marks[key][kn]

        emitting = which is not None

        # ---------------- preloads (sync)
        if which == "sync":
            for dst_t, src_t in ((sb_idx, idx_p), (sb_wedge, wedge_p),
                                 (sb_diag, diag_p), (xnm_sb, xnm_p),
                                 (w1_sb, w1_p), (w2_sb, w2_p), (bias_sb, bias_p),
                                 (fc1w_sb, fc1w_p), (fc1b_sb, fc1b_p),
                                 (fc2w_sb, fc2w_p), (fc2b_sb, fc2b_p),
                                 (ident_sb, ident_p)):
                inc("hw", eng.dma_start(out=dst_t[:], in_=src_t[:]))
        else:
            for _ in range(PRELOADS):
                inc("hw")
        mark("preload")

        for l in range(m.L):
            Fin = m.F if l == 0 else m.H
            wsrc = w1_sb if l == 0 else w2_sb
            wcol0 = 0 if l == 0 else (l - 1) * m.K * m.H
            h_nm = xnm_sb if l == 0 else nmh
            FB = m.F if l == 0 else m.H      # col-block width of h_nm

            # ===== k=0 term: fm = (W_l0)^T @ h  via transpose+mm (initializes fm)
            t0 = l * (m.K - 1)  # prop index of k=1 (for buffer bookkeeping)
            for ci, (g0, ng) in enumerate(m.chunks):
                nn = ng * P
                if which == "tensor":
                    if l > 0:
                        wait("a", mget(("a_nmh", l - 1), "a"))
                    wait("a", mget(("a_tk", l, 0, ci - 2), "a"))  # psT reuse
                    last = None
                    for gg in range(ng):
                        last = eng.matmul(
                            out=psT[ci % 2][:Fin, gg * P:(gg + 1) * P],
                            lhsT=h_nm[:, (g0 + gg) * FB:(g0 + gg) * FB + Fin],
                            rhs=ident_sb[:, :],
                            is_transpose=True, start=True, stop=True)
                    inc("p", last)
                else:
                    inc("p")
                mark(("p_tr", l, 0, ci))
                if which == "scalar":
                    wait("p", mget(("p_tr", l, 0, ci), "p"))
                    wait("p", mget(("p_mm", l, 0, ci - 2), "p"))  # tkfm reuse
                    inc("a", eng.activation(out=tkfm[ci % 2][:Fin, :nn],
                                            in_=psT[ci % 2][:Fin, :nn], func=COPY))
                else:
                    inc("a")
                mark(("a_tk", l, 0, ci))
                if which == "tensor":
                    wait("a", mget(("a_tk", l, 0, ci), "a"))
                    # psB reuse: previous layer's k=4 DVE add of this bank
                    wait("v", mget(("v_add", l - 1, m.K - 1, ci), "v"))
                    inc("p", eng.matmul(out=psB[ci % 2][:m.H, :nn],
                                        lhsT=wsrc[:Fin, wcol0:wcol0 + m.H],
                                        rhs=tkfm[ci % 2][:Fin, :nn],
                                        start=True, stop=True))
                else:
                    inc("p")
                mark(("p_mm", l, 0, ci))
                if which == "scalar":
                    wait("p", mget(("p_mm", l, 0, ci), "p"))
                    inc("a", eng.activation(out=fm[:, g0 * P:g0 * P + nn],
                                            in_=psB[ci % 2][:m.H, :nn], func=COPY))
                else:
                    inc("a")
                mark(("a_fm0", l, ci))

            # ===== props k=1..K-1
            for k in range(1, m.K):
                t = l * (m.K - 1) + (k - 1)
                nm_new = nm[t % 3]
                nm_in = h_nm if k == 1 else nm[(t - 1) % 3]
                in_FB = FB if k == 1 else m.H
                nm_prev = (h_nm if k == 2 else nm[(t - 2) % 3]) if k >= 2 else None
                prev_FB = FB if k == 2 else m.H
                if k == 1:
                    tab_in = x_table_p if l == 0 else tabs[0]
                else:
                    tab_in = tabs[k - 1]
                cc_need = n["cc"]

                # ---- gathers (gpsimd)
                # per-prop: parity sems are quiescent here (cc chain implies
                # all prior gathers consumed); clear so 16-bit wait fields
                # stay small.
                gk = lambda ti: f"g{ti % 2}_{t % 10}"
                if which == "gpsimd":
                    wait("cc", cc_need)
                for ti, (c0, c1, segs) in enumerate(m.tiles):
                    cols = c1 - c0
                    if which == "gpsimd":
                        if ti == 1:
                            if cc_need > 0:
                                wait("cc", cc_need)
                            else:
                                wait("hw", marks["preload"]["hw"])
                        if ti >= 2:
                            wait("v", mget(("v_tile", t, ti - 2), "v"))
                        for c in range(c0, c1):
                            if VARIANT == "nogather":
                                inc(gk(ti))
                                continue
                            inc(gk(ti), eng.indirect_dma_start(
                                out=gbuf[ti % 2][:, c - c0, :Fin],
                                out_offset=None,
                                in_=tab_in[:],
                                in_offset=IndirectOffsetOnAxis(
                                    ap=sb_idx[:, c:c + 1], axis=0)))
                    else:
                        for _ in range(cols):
                            inc(gk(ti))
                    mark(("g_tile", t, ti))

                # ---- DVE per tile: wedge mul + slot reduce
                for ti, (c0, c1, segs) in enumerate(m.tiles):
                    cols = c1 - c0
                    if which == "vector":
                        if ti == 0:
                            # nm_new buffer reuse: transposes of prop t-3 done
                            wait("p", mget(("p_tr_done", t - 3), "p"))
                        if VARIANT != "nogather":
                            wait(gk(ti), mget(("g_tile", t, ti), gk(ti)))
                        gb = gbuf[ti % 2]
                        if VARIANT == "nodve":
                            inc("v", eng.tensor_copy(out=gb[:, 0, :2],
                                                     in_=gb[:, 0, :2]))
                            mark(("v_tile", t, ti))
                            continue
                        eng.tensor_tensor(
                            out=gb[:, :cols, :Fin], in0=gb[:, :cols, :Fin],
                            in1=sb_wedge[:, c0:c1, None].to_broadcast([P, cols, Fin]),
                            op=MULT)
                        eng.drain()
                        last = None
                        for (crel, gstart, ngg, D) in segs:
                            src_ap = gb[:, crel:crel + ngg * D, :Fin].rearrange(
                                "p (g d) f -> p g f d", d=D)
                            dst_ap = nm_new[:, gstart * m.H:(gstart + ngg) * m.H] \
                                .rearrange("p (g f) -> p g f", f=m.H)[:, :, :Fin]
                            last = eng.tensor_reduce(out=dst_ap, in_=src_ap,
                                                     axis=X, op=ADD)
                        inc("v", last)
                    else:
                        inc("v")
                    mark(("v_tile", t, ti))

                # ---- combine
                if which == "vector" and VARIANT == "nodve":
                    inc("v", eng.tensor_copy(out=nm_new[:, :2], in_=nm_new[:, :2]))
                elif which == "vector":
                    eng.drain()
                    last = None
                    for g in range(m.NGRP):
                        last = eng.scalar_tensor_tensor(
                            out=nm_new[:, g * m.H:g * m.H + Fin],
                            in0=nm_in[:, g * in_FB:g * in_FB + Fin],
                            scalar=sb_diag[:, g:g + 1],
                            in1=nm_new[:, g * m.H:g * m.H + Fin],
                            op0=MULT, op1=ADD)
                    if k > 1:
                        eng.drain()
                        for g in range(m.NGRP):
                            last = eng.scalar_tensor_tensor(
                                out=nm_new[:, g * m.H:g * m.H + Fin],
                                in0=nm_new[:, g * m.H:g * m.H + Fin],
                                scalar=2.0,
                                in1=nm_prev[:, g * prev_FB:g * prev_FB + Fin],
                                op0=MULT, op1=SUB)
                    inc("v", last)
                else:
                    inc("v")
                mark(("v_comb", t))

                if DBG and t == 0:
                    if which == "sync":
                        wait("v", mget(("v_comb", t), "v"))
                        inc("hw", eng.dma_start(out=dbg0_p[:], in_=nm_new[:, :]))
                    else:
                        inc("hw")

                # ---- slice write + allgather (k <= K-2)
                if k < m.K - 1:
                    sl = slc[t % 2]
                    tb = tabs[k]
                    if which == "gpsimd":
                        wait("v", mget(("v_comb", t), "v"))
                        inc("s", eng.dma_start(
                            out=sl[:].rearrange("(g p) f -> p g f", p=P)[:, :, :Fin],
                            in_=nm_new[:, :].rearrange(
                                "p (g f) -> p g f", f=m.H)[:, :, :Fin]))
                    else:
                        inc("s")
                    mark(("hw_slc", t))
                    if which == "gpsimd":
                        wait("s", mget(("hw_slc", t), "s"))
                        inc("cc", eng.collective_compute(
                            "AllGather", mybir.AluOpType.bypass,
                            replica_groups=[core_ids],
                            ins=[sl[:]], outs=[tb[:]]))
                    else:
                        inc("cc")
                    mark(("cc", t))

                # ---- W_lk accumulation
                for ci, (g0, ng) in enumerate(m.chunks):
                    nn = ng * P
                    if which == "tensor":
                        wait("v", mget(("v_comb", t), "v"))
                        wait("a", mget(("a_tk", l, k, ci - 2), "a"))
                        last = None
                        for gg in range(ng):
                            last = eng.matmul(
                                out=psT[ci % 2][:Fin, gg * P:(gg + 1) * P],
                                lhsT=nm_new[:, (g0 + gg) * m.H:(g0 + gg) * m.H + Fin],
                                rhs=ident_sb[:, :],
                                is_transpose=True, start=True, stop=True)
                        inc("p", last)
                    else:
                        inc("p")
                    mark(("p_tr", l, k, ci))
                    if which == "scalar":
                        wait("p", mget(("p_tr", l, k, ci), "p"))
                        wait("p", mget(("p_mm", l, k, ci - 2), "p"))
                        inc("a", eng.activation(out=tkfm[ci % 2][:Fin, :nn],
                                                in_=psT[ci % 2][:Fin, :nn],
                                                func=COPY))
                    else:
                        inc("a")
                    mark(("a_tk", l, k, ci))
                    if which == "tensor":
                        wait("a", mget(("a_tk", l, k, ci), "a"))
                        wait("v", mget(("v_add", l, k - 1, ci) if k > 1
                                       else ("v_add", l - 1, m.K - 1, ci), "v"))
                        inc("p", eng.matmul(
                            out=psB[ci % 2][:m.H, :nn],
                            lhsT=wsrc[:Fin, wcol0 + k * m.H:wcol0 + (k + 1) * m.H],
                            rhs=tkfm[ci % 2][:Fin, :nn],
                            start=True, stop=True))
                    else:
                        inc("p")
                    mark(("p_mm", l, k, ci))
                    if which == "vector":
                        wait("p", mget(("p_mm", l, k, ci), "p"))
                        wait("a", mget(("a_fm0", l, ci), "a"))
                        inc("v", eng.tensor_add(
                            out=fm[:, g0 * P:g0 * P + nn],
                            in0=fm[:, g0 * P:g0 * P + nn],
                            in1=psB[ci % 2][:m.H, :nn]))
                    else:
                        inc("v")
                    mark(("v_add", l, k, ci))
                mark(("p_tr_done", t))

            # ===== epilogue: relu + transpose h -> nmh
            for ci, (g0, ng) in enumerate(m.chunks):
                nn = ng * P
                if which == "scalar":
                    wait("v", mget(("v_add", l, m.K - 1, ci), "v"))
                    inc("a", eng.activation(out=fm[:, g0 * P:g0 * P + nn],
                                            in_=fm[:, g0 * P:g0 * P + nn],
                                            func=RELU, bias=bias_sb[:, l:l + 1]))
                else:
                    inc("a")
                mark(("a_relu", l, ci))
            for ci, (g0, ng) in enumerate(m.chunks):
                if which == "tensor":
                    wait("a", mget(("a_relu", l, ci), "a"))
                    wait("a", mget(("a_hc", l, ci - 2), "a"))  # psT reuse
                    last = None
                    for gg in range(ng):
                        last = eng.matmul(
                            out=psT[ci % 2][:P, gg * m.H:(gg + 1) * m.H],
                            lhsT=fm[:, (g0 + gg) * P:(g0 + gg + 1) * P],
                            rhs=ident_sb[:m.H, :m.H],
                            is_transpose=True, start=True, stop=True)
                    inc("p", last)
                else:
                    inc("p")
                mark(("p_ht", l, ci))
                if which == "scalar":
                    wait("p", mget(("p_ht", l, ci), "p"))
                    inc("a", eng.activation(
                        out=nmh[:, g0 * m.H:(g0 + ng) * m.H],
                        in_=psT[ci % 2][:P, :ng * m.H], func=COPY))
                else:
                    inc("a")
                mark(("a_hc", l, ci))
            mark(("a_nmh", l))

            if DBG and l == 0:
                if which == "sync":
                    wait("a", mget(("a_nmh", l), "a"))
                    inc("hw", eng.dma_start(out=dbg1_p[:], in_=nmh[:, :]))
                else:
                    inc("hw")

            if l < m.L - 1:
                if which == "gpsimd":
                    wait("a", mget(("a_nmh", l), "a"))
                    inc("s", eng.dma_start(
                        out=slc[l % 2][:].rearrange(
                            "(g p) f -> p g f", p=P)[:, :, :m.H],
                        in_=nmh[:, :].rearrange("p (g f) -> p g f", f=m.H)))
                else:
                    inc("s")
                mark(("hw_h", l))
                if which == "gpsimd":
                    wait("s", mget(("hw_h", l), "s"))
                    inc("cc", eng.collective_compute(
                        "AllGather", mybir.AluOpType.bypass,
                        replica_groups=[core_ids],
                        ins=[slc[l % 2][:]], outs=[tabs[0][:]]))
                else:
                    inc("cc")
                mark(("cc_h", l))

        # ---------------- pooling
        for g in range(m.NGRP):
            if which == "sync":
                if g >= 1:
                    wait("a", mget(("a_nmh", m.L - 1), "a"))  # fusion breaker
                wait("p", mget(("p_pool", g - 2), "p"))
                wait(f"b{g % 2}", n[f"b{g % 2}"])  # self-order sem
                inc(f"b{g % 2}", eng.dma_start(out=bt[g % 2][:, :],
                                        in_=bmat_p[g * P:(g + 1) * P, :]))
            else:
                inc(f"b{g % 2}")
            mark(("hw_b", g))
            if which == "tensor":
                wait("a", mget(("a_nmh", m.L - 1), "a"))
                wait(f"b{g % 2}", mget(("hw_b", g), f"b{g % 2}"))
                inc("p", eng.matmul(out=psP[:m.H, :m.G],
                                    lhsT=nmh[:, g * m.H:(g + 1) * m.H],
                                    rhs=bt[g % 2][:, :],
                                    start=(g == 0), stop=(g == m.NGRP - 1)))
            else:
                inc("p")
            mark(("p_pool", g))

        if which == "scalar":
            wait("p", mget(("p_pool", m.NGRP - 1), "p"))
            inc("a", eng.activation(out=pooled_sb[:, :], in_=psP[:m.H, :m.G],
                                    func=COPY))
        else:
            inc("a")
        mark("a_pool")
        if which == "sync":
            wait("a", mget("a_pool", "a"))
            inc("hw", eng.dma_start(out=pool_in[:], in_=pooled_sb[:]))
        else:
            inc("hw")
        mark("hw_pool")
        if which == "gpsimd":
            wait("hw", mget("hw_pool", "hw"))
            inc("cc", eng.collective_compute(
                "AllReduce", mybir.AluOpType.add, replica_groups=[core_ids],
                ins=[pool_in[:]], outs=[pool_red[:]]))
        else:
            inc("cc")
        mark("cc_pool")
        if which == "sync":
            wait("cc", mget("cc_pool", "cc"))
            inc("hw", eng.dma_start(out=pooled_sb[:], in_=pool_red[:]))
        else:
            inc("hw")
        mark("hw_pool2")

        if which == "tensor":
            wait("hw", mget("hw_pool2", "hw"))
            inc("p", eng.matmul(out=psB[0][:m.FC1, :m.G], lhsT=fc1w_sb[:, :],
                                rhs=pooled_sb[:, :], start=True, stop=True))
        else:
            inc("p")
        mark("p_fc1")
        if which == "scalar":
            wait("p", mget("p_fc1", "p"))
            inc("a", eng.activation(out=fc1_sb[:, :], in_=psB[0][:m.FC1, :m.G],
                                    func=RELU, bias=fc1b_sb[:, :]))
        else:
            inc("a")
        mark("a_fc1")
        if which == "tensor":
            wait("a", mget("a_fc1", "a"))
            inc("p", eng.matmul(out=psB[1][:1, :m.G], lhsT=fc2w_sb[:, :],
                                rhs=fc1_sb[:, :], start=True, stop=True))
        else:
            inc("p")
        mark("p_fc2")
        if which == "vector":
            wait("p", mget("p_fc2", "p"))
            inc("v", eng.tensor_scalar_add(out_sb[:, :], psB[1][:1, :m.G],
                                           fc2b_sb[:, :]))
        else:
            inc("v")
        mark("v_out")
        if DBG:
            if which == "sync":
                wait("a", mget("a_pool", "a"))
                inc("hw", eng.dma_start(out=dbg2_p[:], in_=pooled_sb[:]))
            else:
                inc("hw")
        if which == "sync":
            wait("v", mget("v_out", "v"))
            inc("hw", eng.dma_start(out=out_p[:], in_=out_sb[:]))
        else:
            inc("hw")

    # record pass (twice: second pass resolves forward references)
    emit(None, None, True)
    emit(None, None, True)

    with nc.Block() as block:
        @block.sync
        def _(sync):
            emit(sync, "sync", False)

        @block.gpsimd
        def _(gpsimd):
            from concourse import library_config as _lc
            gpsimd.load_library(_lc.mlp)
            gpsimd.wait_ge(sem_hw, marks["preload"]["hw"])
            emit(gpsimd, "gpsimd", False)

        @block.vector
        def _(vector):
            vector.wait_ge(sem_hw, marks["preload"]["hw"])
            emit(vector, "vector", False)

        @block.tensor
        def _(tensor):
            tensor.wait_ge(sem_hw, marks["preload"]["hw"])
            emit(tensor, "tensor", False)

        @block.scalar
        def _(scalar):
            emit(scalar, "scalar", False)

    ctx.close()
    nc.compile()
    return nc


def make_in_maps(meta, arrs, wts):
    import ml_dtypes
    m = meta

    def _padw(a, w=128):
        out = np.zeros((a.shape[0], w), a.dtype)
        out[:, :a.shape[1]] = a
        return out

    maps = []
    for c in range(m.ncores):
        maps.append(dict(
            x_table=arrs["x_table"].astype(ml_dtypes.bfloat16),
            xnm=arrs["xnm"][c],
            idx=arrs["idx"][c],
            wedge=arrs["wedge"][c].astype(ml_dtypes.bfloat16),
            diag=arrs["diag"][c],
            bmat=arrs["bmat"][c], w1=wts["w1"], w2=wts["w2"], bias=wts["bias"],
            fc1w=wts["fc1w"], fc1b=wts["fc1b"], fc2w=wts["fc2w"],
            fc2b=wts["fc2b"], ident=np.eye(P, dtype=np.float32)))
    return maps


def kernel(**inputs):
    from concourse.bass_utils import run_bass_kernel_spmd
    meta, arrs = preprocess(inputs["x"], inputs["edge_index"], inputs["batch"],
                            inputs["lmax"])
    wts = pack_weights(meta, *[inputs[k] for k in
                               ("W1", "b1", "W2", "b2", "W3", "b3", "W4", "b4",
                                "W5", "b5", "fc1_w", "fc1_b", "fc2_w", "fc2_b")])
    nc = build_nc(meta)
    res = run_bass_kernel_spmd(nc, make_in_maps(meta, arrs, wts),
                               list(range(meta.ncores)))
    return np.asarray(res.results[0]["out"]).reshape(meta.G, 1).astype(np.float32)



# revision 10
# speedup vs baseline: 1.4397x; 1.4397x over previous
"""ChebNet (K=5, 5 conv layers, H=48) forward on 8 TRN2 NeuronCores.

Host prep: relabel nodes (in-degree sorted, round-robin over cores; the output
is graph-pooled so relabeling is free), build per-node in-edge slot tables
padded to per-group quantized degrees, precompute wedge/diag.

Device per prop (20 total): indirect-DMA gather of src rows (192B each) from a
replicated HBM table -> DVE wedge-multiply + slot tensor_reduce -> Chebyshev
combine -> AllGather local slices into the next table.  PE transposes each
T_k to feature-major and accumulates sum_k T_k @ W_k; ACT does PSUM->SBUF
copies and ReLU.  Pooling = one-hot matmul; FCs on device; out = [G,1].
"""

import contextlib
import numpy as np
from dataclasses import dataclass, field

P = 128
QLEV = [4, 6, 8, 12, 16, 24, 32, 48, 64, 96, 128, 192, 256, 384, 512, 768, 1024]


def _quant(d):
    for q in QLEV:
        if q >= d:
            return q
    raise ValueError(d)


@dataclass
class Meta:
    N: int; E: int; G: int; F: int; H: int; K: int; L: int
    ncores: int; NLOC: int; NGRP: int; NLOCP: int; NT: int
    Dg: list; colst: list; TD: int; CMAX: int
    tiles: list = field(default_factory=list)
    chunks: list = field(default_factory=list)
    FC1: int = 32
    CH: int = 32768
    NCHK: int = 4
    gsegs: list = field(default_factory=list)
    toks_per_tile: list = field(default_factory=list)
    TOTTOK: int = 0


def make_tiles(meta: Meta):
    tiles = []
    g = 0
    while g < meta.NGRP:
        c0 = meta.colst[g]
        g1 = g
        while g1 < meta.NGRP and meta.colst[g1 + 1] - c0 <= meta.CMAX:
            g1 += 1
        if g1 == g:       # single oversized group gets its own tile
            g1 = g + 1
        c1 = meta.colst[g1]
        segs = []
        gg = g
        while gg < g1:
            ge = gg
            while ge < g1 and meta.Dg[ge] == meta.Dg[gg]:
                ge += 1
            segs.append((meta.colst[gg] - c0, gg, ge - gg, meta.Dg[gg]))
            gg = ge
        tiles.append((c0, c1, segs))
        g = g1
    meta.tiles = tiles
    chunks = []
    g = 0
    while g < meta.NGRP:
        n = min(4, meta.NGRP - g)
        chunks.append((g, n))
        g += n
    meta.chunks = chunks


def preprocess(x, edge_index, batch, lmax, ncores=8, cmax=128):
    x = np.asarray(x, np.float32)
    src = np.asarray(edge_index[0], np.int64)
    dst = np.asarray(edge_index[1], np.int64)
    batch = np.asarray(batch, np.int64)
    lmax = np.asarray(lmax, np.float32)
    N, F = x.shape
    E = src.shape[0]
    G = lmax.shape[0]

    deg = np.bincount(src, minlength=N).astype(np.float32)
    dis = np.where(deg > 0,
                   1.0 / np.sqrt(np.where(deg > 0, deg, 1.0).astype(np.float32)),
                   0.0).astype(np.float32)
    wedge = (-2.0 * dis[src] * dis[dst] / lmax[batch[src]]).astype(np.float32)
    diag = (2.0 / lmax[batch] - 1.0).astype(np.float32)

    indeg = np.bincount(dst, minlength=N)
    order = np.argsort(-indeg, kind="stable")
    NLOC = (N + ncores - 1) // ncores
    NGRP = (NLOC + P - 1) // P
    NLOCP = NGRP * P
    NT = ncores * NLOCP
    j = np.arange(N)
    new_id = np.empty(N, np.int64)
    new_id[order] = (j % ncores) * NLOCP + j // ncores
    indeg_sorted = indeg[order]

    src_n = new_id[src]
    dst_n = new_id[dst]

    Dg = []
    for g in range(NGRP):
        kq = g * P * ncores
        dmax = int(indeg_sorted[min(kq, N - 1)]) if kq < N else 0
        Dg.append(max(2, ((dmax + 1) // 2) * 2))
    colst = np.concatenate([[0], np.cumsum(Dg)]).astype(np.int64)
    TD = int(colst[-1])

    c_e = dst_n // NLOCP
    ldst = dst_n % NLOCP
    key = c_e * NLOCP + ldst
    ord2 = np.argsort(key, kind="stable")
    sk = key[ord2]
    starts = np.flatnonzero(np.concatenate([[True], sk[1:] != sk[:-1]]))
    counts = np.diff(np.concatenate([starts, [E]]))
    slot = np.arange(E) - np.repeat(starts, counts)
    g_of = (sk % NLOCP) // P
    p_of = sk % P
    c_of = sk // NLOCP
    assert (slot < np.array(Dg)[g_of]).all()
    col = colst[g_of] + slot

    idxa_arr = np.zeros((ncores, P, TD), np.int32)
    wedge_arr = np.zeros((ncores, P, TD), np.float32)
    idxa_arr[c_of, p_of, col] = src_n[ord2].astype(np.int32)
    wedge_arr[c_of, p_of, col] = wedge[ord2]

    diag_arr = np.zeros((ncores, P, NGRP), np.float32)
    cj, rj = j % ncores, j // ncores
    diag_arr[cj, rj % P, rj // P] = diag[order]

    x_table = np.zeros((NT, F), np.float32)
    x_table[new_id] = x
    # x local, node-major blocks: [P, NGRP*F], block g col f = x of node g*128+p
    xnm = np.zeros((ncores, P, NGRP * F), np.float32)
    xl = x_table.reshape(ncores, NGRP, P, F)
    xnm[:] = xl.transpose(0, 2, 1, 3).reshape(ncores, P, NGRP * F)

    bmat = np.zeros((ncores, NLOCP, G), np.float32)
    bnew = np.full(NT, -1, np.int64)
    bnew[new_id] = batch
    for c in range(ncores):
        bl = bnew[c * NLOCP:(c + 1) * NLOCP]
        msk = bl >= 0
        bmat[c][np.flatnonzero(msk), bl[msk]] = 1.0

    meta = Meta(N=N, E=E, G=G, F=F, H=48, K=5, L=5, ncores=ncores, NLOC=NLOC,
                NGRP=NGRP, NLOCP=NLOCP, NT=NT, Dg=Dg, colst=list(map(int, colst)),
                TD=TD, CMAX=cmax)
    make_tiles(meta)
    arrs = dict(idx=idxa_arr, wedge=wedge_arr, diag=diag_arr,
                x_table=x_table, xnm=xnm, bmat=bmat)
    return meta, arrs


def pack_weights(meta, W1, b1, W2, b2, W3, b3, W4, b4, W5, b5,
                 fc1_w, fc1_b, fc2_w, fc2_b):
    K, F, H = np.asarray(W1).shape
    w1p = np.ascontiguousarray(np.asarray(W1, np.float32).transpose(1, 0, 2).reshape(F, K * H))
    w2p = np.concatenate(
        [np.asarray(W, np.float32).transpose(1, 0, 2).reshape(H, K * H)
         for W in (W2, W3, W4, W5)], axis=1)
    bp = np.stack([np.asarray(b, np.float32) for b in (b1, b2, b3, b4, b5)], axis=1)
    return dict(w1=w1p, w2=w2p, bias=bp,
                fc1w=np.asarray(fc1_w, np.float32),
                fc1b=np.asarray(fc1_b, np.float32).reshape(-1, 1),
                fc2w=np.asarray(fc2_w, np.float32).reshape(-1, 1),
                fc2b=np.asarray(fc2_b, np.float32).reshape(1, 1))


# ------------------------------------------------------------------ numpy model
def emulate(meta, arrs, wts):
    m = meta
    idx, wedge, diag = arrs["idx"], arrs["wedge"], arrs["diag"]
    table = arrs["x_table"]
    h_nm = None   # [nc, NLOCP, Fin] local node-major (natural order)
    x_loc = table.reshape(m.ncores, m.NLOCP, m.F)
    h_loc = x_loc
    h = None
    for l in range(m.L):
        Fin = m.F if l == 0 else m.H
        wl = wts["w1"] if l == 0 else wts["w2"][:, (l - 1) * m.K * m.H:l * m.K * m.H]
        out = np.einsum("cnf,fh->cnh", h_loc, wl[:, 0:m.H])
        t_loc = h_loc
        t_prev = None
        cur_table = table
        for k in range(1, m.K):
            gath = cur_table[idx]                              # [nc, P, TD, Fin]
            Y = gath * wedge[..., None]
            red = np.zeros((m.ncores, P, m.NGRP, Fin), np.float32)
            for g in range(m.NGRP):
                red[:, :, g] = Y[:, :, m.colst[g]:m.colst[g + 1]].sum(2)
            tl4 = t_loc.reshape(m.ncores, m.NGRP, P, Fin).transpose(0, 2, 1, 3)
            prop = red + diag[..., None] * tl4
            if k == 1:
                t_new4 = prop
            else:
                tp4 = t_prev.reshape(m.ncores, m.NGRP, P, Fin).transpose(0, 2, 1, 3)
                t_new4 = 2.0 * prop - tp4
            t_new = np.ascontiguousarray(t_new4.transpose(0, 2, 1, 3)).reshape(
                m.ncores, m.NLOCP, Fin)
            out += np.einsum("cnf,fh->cnh", t_new, wl[:, k * m.H:(k + 1) * m.H])
            t_prev, t_loc = t_loc, t_new
            if k < m.K - 1:
                cur_table = t_new.reshape(m.NT, Fin)
        h = np.maximum(out + wts["bias"][:, l], 0.0)
        h_loc = h
        table = h.reshape(m.NT, m.H)
    pooled = np.einsum("cng,cnh->gh", arrs["bmat"], h)
    z = np.maximum(pooled @ wts["fc1w"] + wts["fc1b"].ravel(), 0.0)
    return (z @ wts["fc2w"] + wts["fc2b"].ravel()).astype(np.float32)


# ------------------------------------------------------------------ bass build
def build_nc(meta):
    import concourse.bass as bass
    import concourse.mybir as mybir
    from concourse.bass import IndirectOffsetOnAxis

    F32 = mybir.dt.float32
    BF16 = mybir.dt.bfloat16
    I32 = mybir.dt.int32
    RELU = mybir.ActivationFunctionType.Relu
    COPY = mybir.ActivationFunctionType.Copy
    ADD = mybir.AluOpType.add
    MULT = mybir.AluOpType.mult
    SUB = mybir.AluOpType.subtract
    X = mybir.AxisListType.X

    import concourse.bacc as bacc
    from concourse import library_config

    m = meta
    EW = 128
    nt = len(m.tiles)
    NCH = len(m.chunks)
    NPROP = m.L * (m.K - 1)
    core_ids = list(range(m.ncores))
    nc = bacc.Bacc("TRN2")

    x_table_p = nc.declare_dram_parameter("x_table", [m.NT, m.F], BF16, False)
    xnm_p = nc.declare_dram_parameter("xnm", [P, m.NGRP * m.F], F32, False)
    idx_p = nc.declare_dram_parameter("idx", [P, m.TD], I32, False)
    wedge_p = nc.declare_dram_parameter("wedge", [P, m.TD], BF16, False)
    diag_p = nc.declare_dram_parameter("diag", [P, m.NGRP], F32, False)
    bmat_p = nc.declare_dram_parameter("bmat", [m.NLOCP, m.G], F32, False)
    w1_p = nc.declare_dram_parameter("w1", [m.F, m.K * m.H], F32, False)
    w2_p = nc.declare_dram_parameter("w2", [m.H, (m.L - 1) * m.K * m.H], F32, False)
    bias_p = nc.declare_dram_parameter("bias", [m.H, m.L], F32, False)
    fc1w_p = nc.declare_dram_parameter("fc1w", [m.H, m.FC1], F32, False)
    fc1b_p = nc.declare_dram_parameter("fc1b", [m.FC1, 1], F32, False)
    fc2w_p = nc.declare_dram_parameter("fc2w", [m.FC1, 1], F32, False)
    fc2b_p = nc.declare_dram_parameter("fc2b", [1, 1], F32, False)
    ident_p = nc.declare_dram_parameter("ident", [P, P], F32, False)
    out_p = nc.declare_dram_parameter("out", [1, m.G], F32, True)
    DBG = bool(getattr(m, "debug", False))
    if DBG:
        dbg0_p = nc.declare_dram_parameter("dbg0", [P, m.NGRP * m.H], F32, True)
        dbg1_p = nc.declare_dram_parameter("dbg1", [P, m.NGRP * m.H], F32, True)
        dbg2_p = nc.declare_dram_parameter("dbg2", [m.H, m.G], F32, True)
    PRELOADS = 12

    tabs = [nc.dram_tensor(f"tab{i}", [m.NT, m.H], BF16, addr_space="Shared")
            for i in range(4)]
    slc = [nc.dram_tensor(f"slc{i}", [m.NLOCP, m.H], BF16) for i in range(2)]
    slc_sm = nc.dram_tensor("slc_sm", [128, 16], BF16)
    tab_sm = nc.dram_tensor("tab_sm", [1024, 16], BF16, addr_space="Shared")
    pool_in = nc.dram_tensor("pool_in", [m.H, m.G], F32)
    pool_red = nc.dram_tensor("pool_red", [m.H, m.G], F32, addr_space="Shared")

    ctx = contextlib.ExitStack()
    _cnt = [0]

    def sb(shape, dt=F32):
        _cnt[0] += 1
        return ctx.enter_context(nc.sbuf_tensor(f"sb{_cnt[0]}", shape, dt))

    def ps(shape):
        _cnt[0] += 1
        return ctx.enter_context(nc.psum_tensor(f"ps{_cnt[0]}", shape, F32))

    sb_idx = sb([P, m.TD], I32)
    sb_wedge = sb([P, m.TD], BF16)
    sb_diag = sb([P, m.NGRP])
    gmax = max(c1 - c0 for c0, c1, _ in m.tiles)
    gbuf = [sb([P, gmax, m.H], BF16) for _ in range(2)]
    nm = [sb([P, m.NGRP * m.H]) for _ in range(3)]
    nmh = sb([P, m.NGRP * m.H])
    xnm_sb = sb([P, m.NGRP * m.F])
    fm = sb([m.H, m.NLOCP])
    tkfm = [sb([m.H, 512]) for _ in range(2)]
    w1_sb = sb([m.F, m.K * m.H])
    w2_sb = sb([m.H, (m.L - 1) * m.K * m.H])
    bias_sb = sb([m.H, m.L])
    fc1w_sb = sb([m.H, m.FC1])
    fc1b_sb = sb([m.FC1, 1])
    fc2w_sb = sb([m.FC1, 1])
    fc2b_sb = sb([1, 1])
    ident_sb = sb([P, P])
    bt = [sb([P, m.G]) for _ in range(2)]
    pooled_sb = sb([m.H, m.G])
    fc1_sb = sb([m.FC1, m.G])
    out_sb = sb([1, m.G])

    psT = [ps([P, 512]) for _ in range(2)]
    psB = [ps([P, 512]) for _ in range(2)]
    psP = ps([P, 512])

    sem_gr = {f"g{p}_{r}": ctx.enter_context(nc.semaphore(f"sem_g{p}_{r}"))
              for p in (0, 1) for r in range(10)}
    sem_b0 = ctx.enter_context(nc.semaphore("sem_b0"))
    sem_b1 = ctx.enter_context(nc.semaphore("sem_b1"))
    sem_hw = ctx.enter_context(nc.semaphore("sem_hw"))
    sem_s = ctx.enter_context(nc.semaphore("sem_s"))
    sem_cc = ctx.enter_context(nc.semaphore("sem_cc"))
    sem_v = ctx.enter_context(nc.semaphore("sem_v"))
    sem_p = ctx.enter_context(nc.semaphore("sem_p"))
    sem_a = ctx.enter_context(nc.semaphore("sem_a"))
    SEMS = {"b0": sem_b0, "b1": sem_b1,
            "hw": sem_hw, "s": sem_s, "cc": sem_cc, "v": sem_v, "p": sem_p,
            "a": sem_a}
    SEMS.update(sem_gr)

    marks = {}
    VARIANT = getattr(m, "variant", "full")
    GB = getattr(m, "gather_batch", 1)

    def emit(eng, which, record):
        n = dict(b0=0, b1=0, hw=0, s=0, cc=0, v=0, p=0, a=0)
        for _p in (0, 1):
            for _r in range(10):
                n[f"g{_p}_{_r}"] = 0

        def inc(kn, inst=None):
            amt = 16 if kn.startswith(("g", "b", "hw", "s")) else 1
            n[kn] += amt
            if inst is not None:
                inst.then_inc(SEMS[kn], amt)

        def wait(kn, val):
            if which and val > 0:
                eng.wait_ge(SEMS[kn], val)

        def mark(key):
            if record:
                marks[key] = dict(n)
            return marks[key]

        def mget(key, kn):
            if key not in marks:  # only during record pass for fwd refs
                return 0
            return marks[key][kn]

        emitting = which is not None

        # ---------------- preloads (sync)
        if which == "sync":
            for dst_t, src_t in ((sb_idx, idx_p), (sb_wedge, wedge_p),
                                 (sb_diag, diag_p), (xnm_sb, xnm_p),
                                 (w1_sb, w1_p), (w2_sb, w2_p), (bias_sb, bias_p),
                                 (fc1w_sb, fc1w_p), (fc1b_sb, fc1b_p),
                                 (fc2w_sb, fc2w_p), (fc2b_sb, fc2b_p),
                                 (ident_sb, ident_p)):
                inc("hw", eng.dma_start(out=dst_t[:], in_=src_t[:]))
        else:
            for _ in range(PRELOADS):
                inc("hw")
        mark("preload")

        for l in range(m.L):
            Fin = m.F if l == 0 else m.H
            wsrc = w1_sb if l == 0 else w2_sb
            wcol0 = 0 if l == 0 else (l - 1) * m.K * m.H
            h_nm = xnm_sb if l == 0 else nmh
            FB = m.F if l == 0 else m.H      # col-block width of h_nm

            # ===== k=0 term: fm = (W_l0)^T @ h  via transpose+mm (initializes fm)
            t0 = l * (m.K - 1)  # prop index of k=1 (for buffer bookkeeping)
            for ci, (g0, ng) in enumerate(m.chunks):
                nn = ng * P
                if which == "tensor":
                    if l > 0:
                        wait("a", mget(("a_nmh", l - 1), "a"))
                    wait("a", mget(("a_tk", l, 0, ci - 2), "a"))  # psT reuse
                    last = None
                    for gg in range(ng):
                        last = eng.matmul(
                            out=psT[ci % 2][:Fin, gg * P:(gg + 1) * P],
                            lhsT=h_nm[:, (g0 + gg) * FB:(g0 + gg) * FB + Fin],
                            rhs=ident_sb[:, :],
                            is_transpose=True, start=True, stop=True)
                    inc("p", last)
                else:
                    inc("p")
                mark(("p_tr", l, 0, ci))
                if which == "scalar":
                    wait("p", mget(("p_tr", l, 0, ci), "p"))
                    wait("p", mget(("p_mm", l, 0, ci - 2), "p"))  # tkfm reuse
                    inc("a", eng.activation(out=tkfm[ci % 2][:Fin, :nn],
                                            in_=psT[ci % 2][:Fin, :nn], func=COPY))
                else:
                    inc("a")
                mark(("a_tk", l, 0, ci))
                if which == "tensor":
                    wait("a", mget(("a_tk", l, 0, ci), "a"))
                    # psB reuse: previous layer's k=4 DVE add of this bank
                    wait("v", mget(("v_add", l - 1, m.K - 1, ci), "v"))
                    inc("p", eng.matmul(out=psB[ci % 2][:m.H, :nn],
                                        lhsT=wsrc[:Fin, wcol0:wcol0 + m.H],
                                        rhs=tkfm[ci % 2][:Fin, :nn],
                                        start=True, stop=True))
                else:
                    inc("p")
                mark(("p_mm", l, 0, ci))
                if which == "scalar":
                    wait("p", mget(("p_mm", l, 0, ci), "p"))
                    inc("a", eng.activation(out=fm[:, g0 * P:g0 * P + nn],
                                            in_=psB[ci % 2][:m.H, :nn], func=COPY))
                else:
                    inc("a")
                mark(("a_fm0", l, ci))

            # ===== props k=1..K-1
            for k in range(1, m.K):
                t = l * (m.K - 1) + (k - 1)
                nm_new = nm[t % 3]
                nm_in = h_nm if k == 1 else nm[(t - 1) % 3]
                in_FB = FB if k == 1 else m.H
                nm_prev = (h_nm if k == 2 else nm[(t - 2) % 3]) if k >= 2 else None
                prev_FB = FB if k == 2 else m.H
                if k == 1:
                    tab_in = x_table_p if l == 0 else tabs[0]
                else:
                    tab_in = tabs[k - 1]
                cc_need = n["cc"]

                # ---- gathers (gpsimd)
                # per-prop: parity sems are quiescent here (cc chain implies
                # all prior gathers consumed); clear so 16-bit wait fields
                # stay small.
                gk = lambda ti: f"g{ti % 2}_{t % 10}"
                if which == "gpsimd":
                    wait("cc", cc_need)
                for ti, (c0, c1, segs) in enumerate(m.tiles):
                    cols = c1 - c0
                    if which == "gpsimd":
                        if ti == 1:
                            if cc_need > 0:
                                wait("cc", cc_need)
                            else:
                                wait("hw", marks["preload"]["hw"])
                        if ti >= 2:
                            wait("v", mget(("v_tile", t, ti - 2), "v"))
                        for cb in range(c0, c1, GB):
                            ce = min(cb + GB, c1)
                            if "nogather" in VARIANT:
                                inc(gk(ti))
                                continue
                            if GB == 1:
                                out_ap = gbuf[ti % 2][:, cb - c0, :Fin]
                            else:
                                out_ap = gbuf[ti % 2][:, cb - c0:ce - c0, :Fin]
                            inc(gk(ti), eng.indirect_dma_start(
                                out=out_ap,
                                out_offset=None,
                                in_=tab_in[:],
                                in_offset=IndirectOffsetOnAxis(
                                    ap=sb_idx[:, cb:ce], axis=0)))
                    else:
                        for cb in range(c0, c1, GB):
                            inc(gk(ti))
                    mark(("g_tile", t, ti))

                # ---- DVE per tile: wedge mul + slot reduce
                for ti, (c0, c1, segs) in enumerate(m.tiles):
                    cols = c1 - c0
                    if which == "vector":
                        if ti == 0:
                            # nm_new buffer reuse: transposes of prop t-3 done
                            wait("p", mget(("p_tr_done", t - 3), "p"))
                        if "nogather" not in VARIANT:
                            wait(gk(ti), mget(("g_tile", t, ti), gk(ti)))
                        gb = gbuf[ti % 2]
                        if "nodve" in VARIANT:
                            inc("v", eng.tensor_copy(out=gb[:, 0, :2],
                                                     in_=gb[:, 0, :2]))
                            mark(("v_tile", t, ti))
                            continue
                        eng.tensor_tensor(
                            out=gb[:, :cols, :Fin], in0=gb[:, :cols, :Fin],
                            in1=sb_wedge[:, c0:c1, None].to_broadcast([P, cols, Fin]),
                            op=MULT)
                        eng.drain()
                        last = None
                        for (crel, gstart, ngg, D) in segs:
                            src_ap = gb[:, crel:crel + ngg * D, :Fin].rearrange(
                                "p (g d) f -> p g f d", d=D)
                            dst_ap = nm_new[:, gstart * m.H:(gstart + ngg) * m.H] \
                                .rearrange("p (g f) -> p g f", f=m.H)[:, :, :Fin]
                            last = eng.tensor_reduce(out=dst_ap, in_=src_ap,
                                                     axis=X, op=ADD)
                        inc("v", last)
                    else:
                        inc("v")
                    mark(("v_tile", t, ti))

                # ---- combine
                if which == "vector" and "nodve" in VARIANT:
                    inc("v", eng.tensor_copy(out=nm_new[:, :2], in_=nm_new[:, :2]))
                elif which == "vector":
                    eng.drain()
                    last = None
                    for g in range(m.NGRP):
                        last = eng.scalar_tensor_tensor(
                            out=nm_new[:, g * m.H:g * m.H + Fin],
                            in0=nm_in[:, g * in_FB:g * in_FB + Fin],
                            scalar=sb_diag[:, g:g + 1],
                            in1=nm_new[:, g * m.H:g * m.H + Fin],
                            op0=MULT, op1=ADD)
                    if k > 1:
                        eng.drain()
                        for g in range(m.NGRP):
                            last = eng.scalar_tensor_tensor(
                                out=nm_new[:, g * m.H:g * m.H + Fin],
                                in0=nm_new[:, g * m.H:g * m.H + Fin],
                                scalar=2.0,
                                in1=nm_prev[:, g * prev_FB:g * prev_FB + Fin],
                                op0=MULT, op1=SUB)
                    inc("v", last)
                else:
                    inc("v")
                mark(("v_comb", t))

                if DBG and t == 0:
                    if which == "sync":
                        wait("v", mget(("v_comb", t), "v"))
                        inc("hw", eng.dma_start(out=dbg0_p[:], in_=nm_new[:, :]))
                    else:
                        inc("hw")

                # ---- slice write + allgather (k <= K-2)
                if k < m.K - 1:
                    sl = slc[t % 2]
                    tb = tabs[k]
                    if which == "gpsimd":
                        wait("v", mget(("v_comb", t), "v"))
                        inc("s", eng.dma_start(
                            out=sl[:].rearrange("(g p) f -> p g f", p=P)[:, :, :Fin],
                            in_=nm_new[:, :].rearrange(
                                "p (g f) -> p g f", f=m.H)[:, :, :Fin]))
                    else:
                        inc("s")
                    mark(("hw_slc", t))
                    if which == "gpsimd":
                        wait("s", mget(("hw_slc", t), "s"))
                        if "nocoll" in VARIANT:
                            inc("cc", eng.nop())
                        elif "smallcoll" in VARIANT:
                            inc("cc", eng.collective_compute(
                                "AllGather", mybir.AluOpType.bypass,
                                replica_groups=[core_ids],
                                ins=[slc_sm[:]], outs=[tab_sm[:]]))
                        else:
                            inc("cc", eng.collective_compute(
                                "AllGather", mybir.AluOpType.bypass,
                                replica_groups=[core_ids],
                                ins=[sl[:]], outs=[tb[:]]))
                    else:
                        inc("cc")
                    mark(("cc", t))

                # ---- W_lk accumulation
                for ci, (g0, ng) in enumerate(m.chunks):
                    nn = ng * P
                    if which == "tensor":
                        wait("v", mget(("v_comb", t), "v"))
                        wait("a", mget(("a_tk", l, k, ci - 2), "a"))
                        last = None
                        for gg in range(ng):
                            last = eng.matmul(
                                out=psT[ci % 2][:Fin, gg * P:(gg + 1) * P],
                                lhsT=nm_new[:, (g0 + gg) * m.H:(g0 + gg) * m.H + Fin],
                                rhs=ident_sb[:, :],
                                is_transpose=True, start=True, stop=True)
                        inc("p", last)
                    else:
                        inc("p")
                    mark(("p_tr", l, k, ci))
                    if which == "scalar":
                        wait("p", mget(("p_tr", l, k, ci), "p"))
                        wait("p", mget(("p_mm", l, k, ci - 2), "p"))
                        inc("a", eng.activation(out=tkfm[ci % 2][:Fin, :nn],
                                                in_=psT[ci % 2][:Fin, :nn],
                                                func=COPY))
                    else:
                        inc("a")
                    mark(("a_tk", l, k, ci))
                    if which == "tensor":
                        wait("a", mget(("a_tk", l, k, ci), "a"))
                        wait("v", mget(("v_add", l, k - 1, ci) if k > 1
                                       else ("v_add", l - 1, m.K - 1, ci), "v"))
                        inc("p", eng.matmul(
                            out=psB[ci % 2][:m.H, :nn],
                            lhsT=wsrc[:Fin, wcol0 + k * m.H:wcol0 + (k + 1) * m.H],
                            rhs=tkfm[ci % 2][:Fin, :nn],
                            start=True, stop=True))
                    else:
                        inc("p")
                    mark(("p_mm", l, k, ci))
                    if which == "vector":
                        wait("p", mget(("p_mm", l, k, ci), "p"))
                        wait("a", mget(("a_fm0", l, ci), "a"))
                        inc("v", eng.tensor_add(
                            out=fm[:, g0 * P:g0 * P + nn],
                            in0=fm[:, g0 * P:g0 * P + nn],
                            in1=psB[ci % 2][:m.H, :nn]))
                    else:
                        inc("v")
                    mark(("v_add", l, k, ci))
                mark(("p_tr_done", t))

            # ===== epilogue: relu + transpose h -> nmh
            for ci, (g0, ng) in enumerate(m.chunks):
                nn = ng * P
                if which == "scalar":
                    wait("v", mget(("v_add", l, m.K - 1, ci), "v"))
                    inc("a", eng.activation(out=fm[:, g0 * P:g0 * P + nn],
                                            in_=fm[:, g0 * P:g0 * P + nn],
                                            func=RELU, bias=bias_sb[:, l:l + 1]))
                else:
                    inc("a")
                mark(("a_relu", l, ci))
            for ci, (g0, ng) in enumerate(m.chunks):
                if which == "tensor":
                    wait("a", mget(("a_relu", l, ci), "a"))
                    wait("a", mget(("a_hc", l, ci - 2), "a"))  # psT reuse
                    last = None
                    for gg in range(ng):
                        last = eng.matmul(
                            out=psT[ci % 2][:P, gg * m.H:(gg + 1) * m.H],
                            lhsT=fm[:, (g0 + gg) * P:(g0 + gg + 1) * P],
                            rhs=ident_sb[:m.H, :m.H],
                            is_transpose=True, start=True, stop=True)
                    inc("p", last)
                else:
                    inc("p")
                mark(("p_ht", l, ci))
                if which == "scalar":
                    wait("p", mget(("p_ht", l, ci), "p"))
                    inc("a", eng.activation(
                        out=nmh[:, g0 * m.H:(g0 + ng) * m.H],
                        in_=psT[ci % 2][:P, :ng * m.H], func=COPY))
                else:
                    inc("a")
                mark(("a_hc", l, ci))
            mark(("a_nmh", l))

            if DBG and l == 0:
                if which == "sync":
                    wait("a", mget(("a_nmh", l), "a"))
                    inc("hw", eng.dma_start(out=dbg1_p[:], in_=nmh[:, :]))
                else:
                    inc("hw")

            if l < m.L - 1:
                if which == "gpsimd":
                    wait("a", mget(("a_nmh", l), "a"))
                    inc("s", eng.dma_start(
                        out=slc[l % 2][:].rearrange(
                            "(g p) f -> p g f", p=P)[:, :, :m.H],
                        in_=nmh[:, :].rearrange("p (g f) -> p g f", f=m.H)))
                else:
                    inc("s")
                mark(("hw_h", l))
                if which == "gpsimd":
                    wait("s", mget(("hw_h", l), "s"))
                    if "nocoll" in VARIANT:
                        inc("cc", eng.nop())
                    elif "smallcoll" in VARIANT:
                        inc("cc", eng.collective_compute(
                            "AllGather", mybir.AluOpType.bypass,
                            replica_groups=[core_ids],
                            ins=[slc_sm[:]], outs=[tab_sm[:]]))
                    else:
                        inc("cc", eng.collective_compute(
                            "AllGather", mybir.AluOpType.bypass,
                            replica_groups=[core_ids],
                            ins=[slc[l % 2][:]], outs=[tabs[0][:]]))
                else:
                    inc("cc")
                mark(("cc_h", l))

        # ---------------- pooling
        for g in range(m.NGRP):
            if which == "sync":
                if g >= 1:
                    wait("a", mget(("a_nmh", m.L - 1), "a"))  # fusion breaker
                wait("p", mget(("p_pool", g - 2), "p"))
                wait(f"b{g % 2}", n[f"b{g % 2}"])  # self-order sem
                inc(f"b{g % 2}", eng.dma_start(out=bt[g % 2][:, :],
                                        in_=bmat_p[g * P:(g + 1) * P, :]))
            else:
                inc(f"b{g % 2}")
            mark(("hw_b", g))
            if which == "tensor":
                wait("a", mget(("a_nmh", m.L - 1), "a"))
                wait(f"b{g % 2}", mget(("hw_b", g), f"b{g % 2}"))
                inc("p", eng.matmul(out=psP[:m.H, :m.G],
                                    lhsT=nmh[:, g * m.H:(g + 1) * m.H],
                                    rhs=bt[g % 2][:, :],
                                    start=(g == 0), stop=(g == m.NGRP - 1)))
            else:
                inc("p")
            mark(("p_pool", g))

        if which == "scalar":
            wait("p", mget(("p_pool", m.NGRP - 1), "p"))
            inc("a", eng.activation(out=pooled_sb[:, :], in_=psP[:m.H, :m.G],
                                    func=COPY))
        else:
            inc("a")
        mark("a_pool")
        if which == "sync":
            wait("a", mget("a_pool", "a"))
            inc("hw", eng.dma_start(out=pool_in[:], in_=pooled_sb[:]))
        else:
            inc("hw")
        mark("hw_pool")
        if which == "gpsimd":
            wait("hw", mget("hw_pool", "hw"))
            if "nocoll" in VARIANT:
                inc("cc", eng.nop())
            else:
                inc("cc", eng.collective_compute(
                    "AllReduce", mybir.AluOpType.add, replica_groups=[core_ids],
                    ins=[pool_in[:]], outs=[pool_red[:]]))
        else:
            inc("cc")
        mark("cc_pool")
        if which == "sync":
            wait("cc", mget("cc_pool", "cc"))
            inc("hw", eng.dma_start(out=pooled_sb[:], in_=pool_red[:]))
        else:
            inc("hw")
        mark("hw_pool2")

        if which == "tensor":
            wait("hw", mget("hw_pool2", "hw"))
            inc("p", eng.matmul(out=psB[0][:m.FC1, :m.G], lhsT=fc1w_sb[:, :],
                                rhs=pooled_sb[:, :], start=True, stop=True))
        else:
            inc("p")
        mark("p_fc1")
        if which == "scalar":
            wait("p", mget("p_fc1", "p"))
            inc("a", eng.activation(out=fc1_sb[:, :], in_=psB[0][:m.FC1, :m.G],
                                    func=RELU, bias=fc1b_sb[:, :]))
        else:
            inc("a")
        mark("a_fc1")
        if which == "tensor":
            wait("a", mget("a_fc1", "a"))
            inc("p", eng.matmul(out=psB[1][:1, :m.G], lhsT=fc2w_sb[:, :],
                                rhs=fc1_sb[:, :], start=True, stop=True))
        else:
            inc("p")
        mark("p_fc2")
        if which == "vector":
            wait("p", mget("p_fc2", "p"))
            inc("v", eng.tensor_scalar_add(out_sb[:, :], psB[1][:1, :m.G],
                                           fc2b_sb[:, :]))
        else:
            inc("v")
        mark("v_out")
        if DBG:
            if which == "sync":
                wait("a", mget("a_pool", "a"))
                inc("hw", eng.dma_start(out=dbg2_p[:], in_=pooled_sb[:]))
            else:
                inc("hw")
        if which == "sync":
            wait("v", mget("v_out", "v"))
            inc("hw", eng.dma_start(out=out_p[:], in_=out_sb[:]))
        else:
            inc("hw")

    # record pass (twice: second pass resolves forward references)
    emit(None, None, True)
    emit(None, None, True)

    with nc.Block() as block:
        @block.sync
        def _(sync):
            emit(sync, "sync", False)

        @block.gpsimd
        def _(gpsimd):
            from concourse import library_config as _lc
            gpsimd.load_library(_lc.mlp)
            gpsimd.wait_ge(sem_hw, marks["preload"]["hw"])
            emit(gpsimd, "gpsimd", False)

        @block.vector
        def _(vector):
            vector.wait_ge(sem_hw, marks["preload"]["hw"])
            emit(vector, "vector", False)

        @block.tensor
        def _(tensor):
            tensor.wait_ge(sem_hw, marks["preload"]["hw"])
            emit(tensor, "tensor", False)

        @block.scalar
        def _(scalar):
            emit(scalar, "scalar", False)

    ctx.close()
    nc.compile()
    return nc


def make_in_maps(meta, arrs, wts):
    import ml_dtypes
    m = meta

    def _padw(a, w=128):
        out = np.zeros((a.shape[0], w), a.dtype)
        out[:, :a.shape[1]] = a
        return out

    maps = []
    for c in range(m.ncores):
        maps.append(dict(
            x_table=arrs["x_table"].astype(ml_dtypes.bfloat16),
            xnm=arrs["xnm"][c],
            idx=arrs["idx"][c],
            wedge=arrs["wedge"][c].astype(ml_dtypes.bfloat16),
            diag=arrs["diag"][c],
            bmat=arrs["bmat"][c], w1=wts["w1"], w2=wts["w2"], bias=wts["bias"],
            fc1w=wts["fc1w"], fc1b=wts["fc1b"], fc2w=wts["fc2w"],
            fc2b=wts["fc2b"], ident=np.eye(P, dtype=np.float32)))
    return maps


def kernel(**inputs):
    from concourse.bass_utils import run_bass_kernel_spmd
    meta, arrs = preprocess(inputs["x"], inputs["edge_index"], inputs["batch"],
                            inputs["lmax"])
    wts = pack_weights(meta, *[inputs[k] for k in
                               ("W1", "b1", "W2", "b2", "W3", "b3", "W4", "b4",
                                "W5", "b5", "fc1_w", "fc1_b", "fc2_w", "fc2_b")])
    nc = build_nc(meta)
    res = run_bass_kernel_spmd(nc, make_in_maps(meta, arrs, wts),
                               list(range(meta.ncores)))
    return np.asarray(res.results[0]["out"]).reshape(meta.G, 1).astype(np.float32)

